# revision 32
# baseline (speedup 1.0000x reference)
"""MoE-routing actor kernel for 8 Trainium2 NeuronCores.

Strategy (pure data parallel, expert-sorted, uint8 inputs, int8 output):
  - Host: fc1 trunk + relu on BLAS; rows dealt per-expert round-robin to the
    8 cores (shared SPMD graph); per-expert capacities trimmed so each core
    is exactly 32 supers of 1024 rows (overflow rows + mask columns beyond
    the 128 PSUM width are computed exactly on host).
  - Inputs: x is quantized to uint8 with a per-h step (x >= 0 after relu;
    0..255 are exact in bf16); the step is folded into the expert weights.
    The SWDGE (gpsimd) DMA casts uint8 -> bf16 in flight, so DRAM load
    traffic halves vs bf16.  Output is int8 with per-(expert, column) scales
    from a 32k-row sample (margin 1.4); host dequant applies scale AND bias
    (no ones-row on device).  err ~1.3e-2 unmasked (gate 2e-2).
  - Device (raw bacc): per 1024-row super, expert matmuls alternate PE
    partition strips 0/64 (concurrent sub-arrays). PSUM->int8 casts
    alternate DVE/ACT per super -- the binding resource (~18us / 32 supers).
  - DMA: all queues share one ~300 B/ns wire, so global transfer order ==
    consumption order.  gpsimd/SWDGE: supers 0-1 (two slot descs, gating
    the first matmuls), then supers 2-31 in ramped segments, then 5 store
    pairs and the final single-super store.  sync: four 34-partition weff
    pieces (first-needed experts first), then 10 store pairs + one single.
    Total DRAM traffic ~5.5MB/core, under the cast wall.
  - No final completion wait: the walrus block-exit drain fences the rings
    during the semaphore-reset epilogue, hiding the last store's receipt.
"""

import os
import sys

sys.path.insert(0, "/opt/trn_rl_repo")

import numpy as np
import ml_dtypes

BF16 = ml_dtypes.bfloat16

B = 262144
NCORES = 8
J = 16
M = 12
H = 34
S_DIM = 32  # state dim
A = J * J  # 256 action logits
NEG = np.float32(-1.0e9)
SUPER = 1024  # rows per compute chunk
HALF = 512  # PSUM-bank / matmul free-dim granule
NP = 4  # psum ring depth (supers)

N_SUPER = 32
# load segments: the sync HWDGE ring (fast, idle before stores begin)
# carries the early supers in small chunks; the gpsimd SWDGE queue (slower,
# but its desc-gen engine is free) streams the big late segments.
SY_SEGS = ((2, 3), (4, 5), (6, 7), (8, 9))
SW_SEGS = (
    (10, 11, 12, 13), (14, 15, 16, 17, 18),
    (19, 20, 21, 22, 23, 24), (25, 26, 27, 28, 29, 30, 31),
)

_BUILD_CACHE: dict = {}
LAST_RESULT = None  # BassKernelResults of the most recent run (for profiling)


def _make_runs(caps, R):
    """Per 512-row half-chunk, the (expert, row0, row1) runs covering it."""
    offs = np.concatenate([[0], np.cumsum(caps)])
    assert offs[-1] == R
    runs = [[] for _ in range(R // HALF)]
    for m in range(len(caps)):
        lo, hi = int(offs[m]), int(offs[m + 1])
        if lo >= hi:
            continue
        for g in range(lo // HALF, (hi - 1) // HALF + 1):
            a = max(lo, g * HALF)
            b = min(hi, (g + 1) * HALF)
            if a < b:
                runs[g].append((m, a, b))
    return runs


def _segments():
    """(name, supers) list in super order."""
    segs = [("head", (0, 1))]
    for i, s in enumerate(SY_SEGS):
        segs.append((f"sy{i}", s))
    for i, s in enumerate(SW_SEGS):
        segs.append((f"sw{i}", s))
    return segs


def _build(R: int, caps: tuple, Adev: int):
    """Raw-bacc device graph: manual semaphores, static SBUF allocation."""
    from concourse import bacc, mybir

    n_half = R // HALF
    n_super = n_half // 2
    assert n_super == N_SUPER
    runs = _make_runs(list(caps), R)
    f32 = mybir.dt.float32
    bf16 = mybir.dt.bfloat16
    i8 = mybir.dt.int8
    nc = bacc.Bacc("TRN2", target_bir_lowering=False, debug=False)

    # experts needed by supers 0-1 -> first weff piece
    eA = 1 + max(m for g in range(4) for (m, _, _) in runs[g])
    eA = min(eA, M)

    segs = _segments()
    sup2seg = {}
    seg_cbase = []
    c = 0
    for si, (_, sups) in enumerate(segs):
        seg_cbase.append(c)
        for j, s in enumerate(sups):
            assert s == min(sups) + j
            sup2seg[s] = (si, j)
        c += len(sups) * HALF
    assert c == n_super * HALF

    n_pair = n_super // 2

    # DRAM parameters ------------------------------------------------------
    weff_d = nc.declare_dram_parameter("weff", [2, H, M * Adev], bf16,
                                       isOutput=False)
    xat_ds = {}
    for name, sups in segs:
        xat_ds[name] = nc.declare_dram_parameter(
            f"xat_{name}", [2, H, len(sups) * HALF], bf16, isOutput=False
        )
    out_d = nc.declare_dram_parameter(
        "out", [n_pair, Adev, 2 * SUPER], i8, isOutput=True
    )

    # SBUF / PSUM ----------------------------------------------------------
    xa = nc.alloc_sbuf_tensor("xa_sb", [64 + H, n_super * HALF], bf16)
    weff = nc.alloc_sbuf_tensor("weff_sb", [64 + H, M * Adev], bf16)
    otb = nc.alloc_sbuf_tensor("ot_sb", [Adev, n_super * SUPER], i8)
    ots = [otb[:, s * SUPER : (s + 1) * SUPER] for s in range(n_super)]
    psos = [nc.alloc_psum_tensor(f"pso{k}", [Adev, SUPER], f32) for k in range(NP)]

    # semaphores -----------------------------------------------------------
    NSQ = 4  # rotating store sems (per queue)
    sem_wa = nc.alloc_semaphore("sem_wa")  # weff experts [0,eA)
    sem_wb = nc.alloc_semaphore("sem_wb")  # weff experts [eA,M)
    sem_x0a = nc.alloc_semaphore("sem_x0a")  # supers 0-1 slot0
    sem_x0b = nc.alloc_semaphore("sem_x0b")  # supers 0-1 slot1
    sem_seg = {}  # per-segment sems (slot0 inc 16, slot1 inc 16 -> 32 full)
    for name, _ in segs:
        if name != "head":
            sem_seg[name] = nc.alloc_semaphore(f"sem_{name}")
    sem_mm = nc.alloc_semaphore("sem_mm")
    sem_cv = nc.alloc_semaphore("sem_cv")
    sem_ca = nc.alloc_semaphore("sem_ca")
    sem_oe = [nc.alloc_semaphore(f"sem_oe{k}") for k in range(NSQ)]  # sync stores
    sem_og = [nc.alloc_semaphore(f"sem_og{k}") for k in range(NSQ)]  # swdge stores

    def xslice(name):
        si = [i for i, (n, _) in enumerate(segs) if n == name][0]
        return slice(seg_cbase[si], seg_cbase[si] + len(segs[si][1]) * HALF)

    # cast-engine assignment: DVE takes even supers, ACT odd supers
    dve_rank = {sc: sc // 2 + 1 for sc in range(0, n_super, 2)}
    act_rank = {sc: sc // 2 + 1 for sc in range(1, n_super, 2)}

    def wait_cast_done(eng, k):
        if k in dve_rank:
            eng.wait_ge(sem_cv, dve_rank[k])
        else:
            eng.wait_ge(sem_ca, act_rank[k])

    # store units: the sync ring carries most pairs (it is idle once its
    # load descs are out); the swdge queue picks up late pairs after its
    # load stream drains. Last four supers go out as single-super stores
    # split across both queues (short tail).
    sync_units = [(2 * p, 2) for p in (0, 1, 2, 3, 4, 5, 6, 7, 8, 10, 12)]
    gp_units = [(2 * p, 2) for p in (9, 11, 13)]
    sync_units.append((n_super - 4, 1))
    gp_units.append((n_super - 3, 1))
    sync_units.append((n_super - 2, 1))
    gp_units.append((n_super - 1, 1))

    def store_dst(s0, ns):
        p = s0 // 2
        if ns == 2:
            return out_d[p][:]
        off = (s0 % 2) * SUPER
        return out_d[p][:, off : off + SUPER]

    with nc.Block() as block:

        @block.gpsimd
        def _(g):
            for i in range(len(SW_SEGS)):
                name = f"sw{i}"
                sx = sem_seg[name]
                g.dma_start(xa[0:H, xslice(name)], xat_ds[name][0]).then_inc(sx, 16)
                g.dma_start(xa[64 : 64 + H, xslice(name)], xat_ds[name][1]
                            ).then_inc(sx, 16)
            for i, (s0, ns) in enumerate(gp_units):
                for s in range(s0, s0 + ns):
                    wait_cast_done(g, s)
                so = sem_og[i % NSQ]
                if i >= NSQ:
                    g.wait_ge(so, 16 * (i // NSQ))
                g.dma_start(
                    store_dst(s0, ns), otb[:, s0 * SUPER : (s0 + ns) * SUPER]
                ).then_inc(so, 16)

        @block.tensor
        def _(t):
            t.wait_ge(sem_wa, 16)  # strip-0 weffA piece; strip 1 gated below
            t.wait_ge(sem_x0a, 16)  # supers 0-1 slot0; slot1 gated below
            wb_waited = False
            x0b_waited = False
            seg_h1_waited = set()
            for sc in range(n_super):
                si, j = sup2seg[sc]
                name = segs[si][0]
                if j == 0 and name != "head":
                    # slot0 (strip 0) completes first on the FIFO queue:
                    # gate h0 matmuls on 16, h1 on the full 32 (below)
                    t.wait_ge(sem_seg[name], 16)
                if sc >= NP:
                    wait_cast_done(t, sc - NP)
                pso = psos[sc % NP]
                mms = []
                for h in range(2):
                    base = 0 if h == 0 else 64
                    for (m, a, b) in runs[sc * 2 + h]:
                        if m >= eA and not wb_waited:
                            t.wait_ge(sem_wb, 32)
                            wb_waited = True
                        if h == 1 and not x0b_waited:
                            t.wait_ge(sem_x0b, 16)
                            t.wait_ge(sem_wa, 32)  # strip-1 weffA piece
                            x0b_waited = True
                        if h == 1 and name != "head" and si not in seg_h1_waited:
                            t.wait_ge(sem_seg[name], 32)
                            seg_h1_waited.add(si)
                        c0 = a - sc * SUPER
                        c1 = b - sc * SUPER
                        xcol = seg_cbase[si] + j * HALF
                        mms.append(
                            t.matmul(
                                pso[:, c0:c1],
                                weff[base : base + H, m * Adev : (m + 1) * Adev],
                                xa[
                                    base : base + H,
                                    xcol + c0 - h * HALF : xcol + c1 - h * HALF,
                                ],
                                start=True,
                                stop=True,
                            )
                        )
                mms[-1].then_inc(sem_mm, 1)

        @block.vector
        def _(v):
            for sc in range(0, n_super, 2):
                v.wait_ge(sem_mm, sc + 1)
                v.tensor_copy(ots[sc][:, :], psos[sc % NP][:, :]).then_inc(sem_cv, 1)

        @block.scalar
        def _(s):
            for sc in range(1, n_super, 2):
                s.wait_ge(sem_mm, sc + 1)
                s.copy(ots[sc][:, :], psos[sc % NP][:, :]).then_inc(sem_ca, 1)

        @block.sync
        def _(sy):
            # head + early segments on the fast sync ring, in gate order:
            # mm0 needs head-slot0 + weffA strip 0; the h1 strip needs
            # slot1 + weffA strip 1; then supers 2-3, then weffB (first
            # needed in super 2's h1 strip), then supers 4-9; stores last.
            def seg_descs(name):
                sx = sem_seg[name]
                sy.dma_start(xa[0:H, xslice(name)], xat_ds[name][0]).then_inc(
                    sx, 16
                )
                sy.dma_start(xa[64 : 64 + H, xslice(name)], xat_ds[name][1]
                             ).then_inc(sx, 16)

            sy.dma_start(xa[0:H, xslice("head")], xat_ds["head"][0]).then_inc(
                sem_x0a, 16
            )
            sy.dma_start(weff[0:H, 0 : eA * Adev], weff_d[0][:, 0 : eA * Adev]
                         ).then_inc(sem_wa, 16)
            sy.dma_start(xa[64 : 64 + H, xslice("head")], xat_ds["head"][1]
                        ).then_inc(sem_x0b, 16)
            sy.dma_start(
                weff[64 : 64 + H, 0 : eA * Adev], weff_d[1][:, 0 : eA * Adev]
            ).then_inc(sem_wa, 16)
            seg_descs("sy0")
            if eA < M:
                sy.dma_start(
                    weff[0:H, eA * Adev :], weff_d[0][:, eA * Adev :]
                ).then_inc(sem_wb, 16)
                sy.dma_start(
                    weff[64 : 64 + H, eA * Adev :], weff_d[1][:, eA * Adev :]
                ).then_inc(sem_wb, 16)
            for i in range(1, len(SY_SEGS)):
                seg_descs(f"sy{i}")
            for i, (s0, ns) in enumerate(sync_units):
                for s in range(s0, s0 + ns):
                    wait_cast_done(sy, s)
                so = sem_oe[i % NSQ]
                if i >= NSQ:
                    sy.wait_ge(so, 16 * (i // NSQ))
                sy.dma_start(
                    store_dst(s0, ns), otb[:, s0 * SUPER : (s0 + ns) * SUPER]
                ).then_inc(so, 16)
            # no final completion wait: the block-exit drain fences the rings,
            # so in-flight stores land before the NEFF retires

    nc.compile()
    return nc


def kernel(states, epoch_idx, W1, b1, Wout, bout, mask):
    global LAST_RESULT
    from concourse.bass_utils import run_bass_kernel_spmd

    states = np.asarray(states, dtype=np.float32)
    epoch_idx = np.asarray(epoch_idx, dtype=np.int32)
    W1 = np.asarray(W1, dtype=np.float32)
    b1 = np.asarray(b1, dtype=np.float32)
    Wout = np.asarray(Wout, dtype=np.float32)
    bout = np.asarray(bout, dtype=np.float32)
    mask = np.asarray(mask, dtype=np.int32)

    keep = mask.reshape(A) != 0
    kept_cols = np.nonzero(keep)[0]
    Ak = int(len(kept_cols))
    if Ak == 0:
        return np.full((B, J, J), NEG, np.float32)
    Adev = min(Ak, 128)
    dev_cols = kept_cols[:Adev]
    rem_cols = kept_cols[Adev:]

    # --- shared trunk on host (tiny: ~0.6 GFLOP BLAS) ---
    x = np.maximum(states @ W1.T + b1[None, :], 0.0)  # [B, H] f32
    xb = x.astype(BF16)

    # --- route rows: per expert, deal round-robin across cores ---
    core_idx = [[None] * M for _ in range(NCORES)]
    for m in range(M):
        idx_m = np.nonzero(epoch_idx == m)[0]
        for i in range(NCORES):
            core_idx[i][m] = idx_m[i::NCORES]
    cnt = [[len(core_idx[i][m]) for m in range(M)] for i in range(NCORES)]
    caps = [max(cnt[i][m] for i in range(NCORES)) for m in range(M)]
    R = N_SUPER * SUPER
    excess = sum(caps) - R
    while excess > 0:
        m_big = max(range(M), key=lambda m: caps[m])
        d = min(excess, max(1, excess // M))
        caps[m_big] -= d
        excess -= d
    if excess < 0:
        caps[-1] += -excess
    caps = tuple(caps)
    offs = np.concatenate([[0], np.cumsum(caps)])
    ncap = [[min(cnt[i][m], caps[m]) for m in range(M)] for i in range(NCORES)]

    # --- int8 output scales: per-(expert, column), from sampled bias-free
    # logits (bias is applied on host during dequant) ---
    SAMP = 32768
    MARGIN = 1.4
    rng = np.random.default_rng(12345)
    samp = rng.choice(B, SAMP, replace=False)
    Wdev = Wout[:, dev_cols, :]  # [M, Adev, H]
    scale = np.empty((M, Adev), np.float32)
    for m in range(M):
        rows_s = samp[epoch_idx[samp] == m]
        sl = x[rows_s] @ Wdev[m].T
        scale[m] = np.abs(sl).max(axis=0) * (MARGIN / 127.0)

    # --- effective expert weights: [2 strips, H, M*Adev] (no bias row) ---
    weff1 = np.empty((H, M * Adev), np.float32)
    for m in range(M):
        weff1[:, m * Adev : (m + 1) * Adev] = (Wdev[m] / scale[m][:, None]).T
    weff_bf = np.ascontiguousarray(
        np.broadcast_to(weff1.astype(BF16)[None], (2, H, M * Adev))
    )

    # --- pack per-core transposed activations (bf16, per-segment) ---
    segs = _segments()
    in_maps = []
    for i in range(NCORES):
        packed = np.zeros((R, H), BF16)
        for m in range(M):
            r0 = int(offs[m])
            packed[r0 : r0 + ncap[i][m]] = xb[core_idx[i][m][: caps[m]]]
        pv = packed.reshape(N_SUPER, 2, HALF, H)
        imap = {"weff": weff_bf}
        for name, sups in segs:
            lo = min(sups)
            imap[f"xat_{name}"] = np.ascontiguousarray(
                pv[lo : lo + len(sups)]
                .transpose(1, 3, 0, 2)
                .reshape(2, H, len(sups) * HALF)
            )
        in_maps.append(imap)

    key = (R, caps, Adev)
    nc = _BUILD_CACHE.get(key)
    if nc is None:
        nc = _build(R, caps, Adev)
        _BUILD_CACHE[key] = nc

    # retry: rare transient NRT_EXEC_UNIT_UNRECOVERABLE on fresh NEFFs
    last_err = None
    for _attempt in range(3):
        try:
            res = run_bass_kernel_spmd(nc, in_maps, core_ids=list(range(NCORES)))
            break
        except Exception as e:  # noqa: BLE001
            last_err = e
    else:
        raise last_err
    LAST_RESULT = res

    # --- unpack: [n_pair, Adev, 2048] int8 -> rows, dequantize + bias ---
    out_kept = np.zeros((B, Adev), np.float32)
    bdev = bout[:, dev_cols]  # [M, Adev]
    for i in range(NCORES):
        oc = np.asarray(res.results[i]["out"])
        rows = oc.transpose(0, 2, 1).reshape(-1, Adev)[:R]
        for m in range(M):
            r0 = int(offs[m])
            out_kept[core_idx[i][m][: caps[m]]] = (
                rows[r0 : r0 + ncap[i][m]].astype(np.float32) * scale[m][None, :]
                + bdev[m][None, :]
            )

    out_full = np.full((B, A), NEG, np.float32)
    out_full[:, dev_cols] = out_kept

    # --- host remainder: kept columns beyond the device's 128, plus the
    # few per-core cap-overflow rows (exact f32) ---
    for m in range(M):
        rows_m = np.nonzero(epoch_idx == m)[0]
        if len(rem_cols):
            out_full[rows_m[:, None], rem_cols[None, :]] = (
                x[rows_m] @ Wout[m][rem_cols].T + bout[m][rem_cols][None, :]
            )
        ov = np.concatenate(
            [core_idx[i][m][caps[m] :] for i in range(NCORES)]
        ).astype(np.int64)
        if len(ov):
            out_full[ov[:, None], dev_cols[None, :]] = (
                x[ov] @ Wout[m][dev_cols].T + bout[m][dev_cols][None, :]
            )

    return out_full.reshape(B, J, J)


# revision 33
# speedup vs baseline: 1.1139x; 1.1139x over previous
"""MoE-routing actor kernel for 8 Trainium2 NeuronCores.

Strategy (pure data parallel, expert-sorted, bf16 matmul, int8 output):
  - Host: fc1 trunk + relu on BLAS; rows dealt per-expert round-robin to the
    8 cores (shared SPMD graph); per-expert capacities trimmed so each core
    is exactly 32 supers of 1024 rows (overflow rows + mask columns beyond
    the 128 PSUM width are computed exactly on host).
  - Output is int8 with per-(expert, column) scales estimated from a 32k-row
    sample (margin 1.4); 1/scale is folded into the bf16 expert weights; the
    bias is applied on host during dequant (no ones-row -> less load
    traffic). err ~1.16e-2 unmasked (gate 2e-2).
  - Device (raw bacc): per 1024-row super, expert matmuls alternate PE
    partition strips 0/64 (concurrent sub-arrays). PSUM->int8 casts
    alternate DVE/ACT per super (~18us for 32 supers).
  - DMA reality (measured): the sync HWDGE queue sustains ~172 B/ns (~210
    solo), the gpsimd SWDGE queue ~124, the scalar queue ~50; they contend.
    The kernel is bound by the sync queue draining the int8 stores, so the
    schedule maximizes its productive window:
      * sync ring: weff as two 34-partition descs (no dead partitions),
        then supers 0-1 slot0, then all store pairs. The earlier the first
        cast, the earlier stores flow.
      * scalar ring: supers 0-1 slot1 + supers 2-3 (slow queue, early data).
      * gpsimd/SWDGE: released only after weff has landed (its traffic
        otherwise crushes the sync queue's small head transfers), then
        streams supers 4-31 in ramped groups; takes the s30 single store.
      * final pair stored as two singles, one per queue, to cut the tail.
  - No final completion wait: the walrus block-exit drain fences the rings
    during the semaphore-reset epilogue, hiding the last store's receipt.
"""

import os
import sys

sys.path.insert(0, "/opt/trn_rl_repo")

import numpy as np
import ml_dtypes

BF16 = ml_dtypes.bfloat16

B = 262144
NCORES = 8
J = 16
M = 12
H = 34
S_DIM = 32
A = J * J
NEG = np.float32(-1.0e9)
SUPER = 1024
HALF = 512
NP = 4  # psum ring depth (supers)

GROUPS = (2, 2, 3, 4, 5, 6, 6, 4)  # supers per load group (g0 sync/scalar,
N_SUPER = sum(GROUPS)              # g1 scalar, g2+ gpsimd)

_BUILD_CACHE: dict = {}
LAST_RESULT = None


def _make_runs(caps, R):
    """Per 512-row half-chunk, the (expert, row0, row1) runs covering it."""
    offs = np.concatenate([[0], np.cumsum(caps)])
    assert offs[-1] == R
    runs = [[] for _ in range(R // HALF)]
    for m in range(len(caps)):
        lo, hi = int(offs[m]), int(offs[m + 1])
        if lo >= hi:
            continue
        for g in range(lo // HALF, (hi - 1) // HALF + 1):
            a = max(lo, g * HALF)
            b = min(hi, (g + 1) * HALF)
            if a < b:
                runs[g].append((m, a, b))
    return runs


def _build(R: int, caps: tuple, Adev: int):
    """Raw-bacc device graph: manual semaphores, static SBUF allocation."""
    from concourse import bacc, mybir

    n_half = R // HALF
    n_super = n_half // 2
    assert n_super == N_SUPER
    runs = _make_runs(list(caps), R)
    f32 = mybir.dt.float32
    bf16 = mybir.dt.bfloat16
    i8 = mybir.dt.int8
    nc = bacc.Bacc("TRN2", target_bir_lowering=False, debug=False)

    n_grp = len(GROUPS)
    gbase = [sum(GROUPS[:g]) for g in range(n_grp)]
    cbase = [b * HALF for b in gbase]
    sup2grp = {}
    for g in range(n_grp):
        for j in range(GROUPS[g]):
            sup2grp[gbase[g] + j] = (g, j)
    n_pair = n_super // 2

    xat_ds = [
        nc.declare_dram_parameter(f"xat{g}", [2, H, GROUPS[g] * HALF], bf16,
                                  isOutput=False)
        for g in range(n_grp)
    ]
    # weff: [2 strips, H, M*Adev] -- two 34-partition descs, no dead rows
    weff_d = nc.declare_dram_parameter("weff", [2, H, M * Adev], bf16,
                                       isOutput=False)
    out_d = nc.declare_dram_parameter(
        "out", [n_pair, Adev, 2 * SUPER], i8, isOutput=True
    )

    xa = nc.alloc_sbuf_tensor("xa_sb", [64 + H, n_super * HALF], bf16)
    weff = nc.alloc_sbuf_tensor("weff_sb", [64 + H, M * Adev], bf16)
    otb = nc.alloc_sbuf_tensor("ot_sb", [Adev, n_super * SUPER], i8)
    ots = [otb[:, s * SUPER : (s + 1) * SUPER] for s in range(n_super)]
    psos = [nc.alloc_psum_tensor(f"pso{k}", [Adev, SUPER], f32) for k in range(NP)]

    NSX = 4  # rotating input-load sems
    NSQ = 4  # rotating store sems
    sem_w = nc.alloc_semaphore("sem_w")
    sem_g0a = nc.alloc_semaphore("sem_g0a")
    sem_g0b = nc.alloc_semaphore("sem_g0b")
    sem_x = [nc.alloc_semaphore(f"sem_x{k}") for k in range(NSX)]
    sem_mm = nc.alloc_semaphore("sem_mm")
    sem_cv = nc.alloc_semaphore("sem_cv")
    sem_ca = nc.alloc_semaphore("sem_ca")
    sem_oe = [nc.alloc_semaphore(f"sem_oe{k}") for k in range(NSQ)]
    sem_og = nc.alloc_semaphore("sem_og")

    def gslice(g):
        return slice(cbase[g], cbase[g] + GROUPS[g] * HALF)

    # cast-engine assignment: DVE takes even supers, ACT takes odd supers
    dve_rank = {sc: sc // 2 + 1 for sc in range(0, n_super, 2)}
    act_rank = {sc: sc // 2 + 1 for sc in range(1, n_super, 2)}

    def wait_cast_done(eng, k):
        if k in dve_rank:
            eng.wait_ge(sem_cv, dve_rank[k])
        else:
            eng.wait_ge(sem_ca, act_rank[k])

    with nc.Block() as block:

        @block.gpsimd
        def _(g):
            # hold the SWDGE stream until the head pieces (weff on sync)
            # have transferred -- its traffic otherwise crushes them
            g.wait_ge(sem_w, 32)
            for gi in range(2, n_grp):
                sx = sem_x[(gi - 1) % NSX]
                if gi - 1 >= NSX:
                    g.wait_ge(sx, 32 * ((gi - 1) // NSX))
                g.dma_start(xa[0:H, gslice(gi)], xat_ds[gi][0]).then_inc(sx, 16)
                g.dma_start(xa[64 : 64 + H, gslice(gi)], xat_ds[gi][1]).then_inc(
                    sx, 16
                )
            # single-super store for s30 on the swdge queue (parallel tail)
            wait_cast_done(g, n_super - 2)
            g.dma_start(
                out_d[n_pair - 1][:, 0:SUPER],
                otb[:, (n_super - 2) * SUPER : (n_super - 1) * SUPER],
            ).then_inc(sem_og, 16)

        @block.tensor
        def _(t):
            t.wait_ge(sem_w, 16)   # strip-0 weff; strip 1 gated below
            t.wait_ge(sem_g0a, 16)  # supers 0-1 slot0; slot1 gated below
            x0b_waited = False
            for sc in range(n_super):
                gi, j = sup2grp[sc]
                if j == 0 and gi > 0:
                    t.wait_ge(sem_x[(gi - 1) % NSX], 32 * ((gi - 1) // NSX + 1))
                if sc >= NP:
                    wait_cast_done(t, sc - NP)
                pso = psos[sc % NP]
                mms = []
                for h in range(2):
                    base = 0 if h == 0 else 64
                    for (m, a, b) in runs[sc * 2 + h]:
                        if h == 1 and not x0b_waited:
                            t.wait_ge(sem_g0b, 16)
                            t.wait_ge(sem_w, 32)  # strip-1 weff
                            x0b_waited = True
                        c0 = a - sc * SUPER
                        c1 = b - sc * SUPER
                        xcol = cbase[gi] + j * HALF
                        mms.append(
                            t.matmul(
                                pso[:, c0:c1],
                                weff[base : base + H, m * Adev : (m + 1) * Adev],
                                xa[
                                    base : base + H,
                                    xcol + c0 - h * HALF : xcol + c1 - h * HALF,
                                ],
                                start=True,
                                stop=True,
                            )
                        )
                mms[-1].then_inc(sem_mm, 1)

        @block.vector
        def _(v):
            for sc in range(0, n_super, 2):
                v.wait_ge(sem_mm, sc + 1)
                v.tensor_copy(ots[sc][:, :], psos[sc % NP][:, :]).then_inc(sem_cv, 1)

        @block.scalar
        def _(s):
            # group-0 high half + group 1 on the scalar HWDGE ring (slow
            # queue, but the data is early and small)
            s.dma_start(xa[64 : 64 + H, gslice(0)], xat_ds[0][1]).then_inc(
                sem_g0b, 16
            )
            s.dma_start(xa[0:H, gslice(1)], xat_ds[1][0]).then_inc(sem_x[0], 16)
            s.dma_start(xa[64 : 64 + H, gslice(1)], xat_ds[1][1]).then_inc(
                sem_x[0], 16
            )
            for sc in range(1, n_super, 2):
                s.wait_ge(sem_mm, sc + 1)
                s.copy(ots[sc][:, :], psos[sc % NP][:, :]).then_inc(sem_ca, 1)

        @block.sync
        def _(sy):
            sy.dma_start(weff[0:H, :], weff_d[0]).then_inc(sem_w, 16)
            sy.dma_start(weff[64 : 64 + H, :], weff_d[1]).then_inc(sem_w, 16)
            sy.dma_start(xa[0:H, gslice(0)], xat_ds[0][0]).then_inc(sem_g0a, 16)
            for p in range(n_pair - 1):
                wait_cast_done(sy, 2 * p)
                wait_cast_done(sy, 2 * p + 1)
                so = sem_oe[p % NSQ]
                if p >= NSQ:
                    sy.wait_ge(so, 16 * (p // NSQ))
                sy.dma_start(
                    out_d[p][:], otb[:, 2 * p * SUPER : (2 * p + 2) * SUPER]
                ).then_inc(so, 16)
            # final single-super store for s31 (s30 went out on swdge)
            wait_cast_done(sy, n_super - 1)
            sy.dma_start(
                out_d[n_pair - 1][:, SUPER : 2 * SUPER],
                otb[:, (n_super - 1) * SUPER : n_super * SUPER],
            ).then_inc(sem_oe[(n_pair - 1) % NSQ], 16)
            # no final completion wait: the block-exit drain fences the rings

    nc.compile()
    return nc


def kernel(states, epoch_idx, W1, b1, Wout, bout, mask):
    global LAST_RESULT
    from concourse.bass_utils import run_bass_kernel_spmd

    states = np.asarray(states, dtype=np.float32)
    epoch_idx = np.asarray(epoch_idx, dtype=np.int32)
    W1 = np.asarray(W1, dtype=np.float32)
    b1 = np.asarray(b1, dtype=np.float32)
    Wout = np.asarray(Wout, dtype=np.float32)
    bout = np.asarray(bout, dtype=np.float32)
    mask = np.asarray(mask, dtype=np.int32)

    keep = mask.reshape(A) != 0
    kept_cols = np.nonzero(keep)[0]
    Ak = int(len(kept_cols))
    if Ak == 0:
        return np.full((B, J, J), NEG, np.float32)
    Adev = min(Ak, 128)
    dev_cols = kept_cols[:Adev]
    rem_cols = kept_cols[Adev:]

    # --- shared trunk on host (tiny: ~0.6 GFLOP BLAS) ---
    x = np.maximum(states @ W1.T + b1[None, :], 0.0)  # [B, H] f32
    xb = x.astype(BF16)

    # --- route rows: per expert, deal round-robin across cores ---
    core_idx = [[None] * M for _ in range(NCORES)]
    for m in range(M):
        idx_m = np.nonzero(epoch_idx == m)[0]
        for i in range(NCORES):
            core_idx[i][m] = idx_m[i::NCORES]
    cnt = [[len(core_idx[i][m]) for m in range(M)] for i in range(NCORES)]
    caps = [max(cnt[i][m] for i in range(NCORES)) for m in range(M)]
    R = N_SUPER * SUPER
    excess = sum(caps) - R
    while excess > 0:
        m_big = max(range(M), key=lambda m: caps[m])
        d = min(excess, max(1, excess // M))
        caps[m_big] -= d
        excess -= d
    if excess < 0:
        caps[-1] += -excess
    caps = tuple(caps)
    offs = np.concatenate([[0], np.cumsum(caps)])
    ncap = [[min(cnt[i][m], caps[m]) for m in range(M)] for i in range(NCORES)]

    # --- int8 output scales from sampled bias-free logits ---
    SAMP = 32768
    MARGIN = 1.4
    rng = np.random.default_rng(12345)
    samp = rng.choice(B, SAMP, replace=False)
    Wdev = Wout[:, dev_cols, :]  # [M, Adev, H]
    scale = np.empty((M, Adev), np.float32)
    for m in range(M):
        rows_s = samp[epoch_idx[samp] == m]
        sl = x[rows_s] @ Wdev[m].T
        scale[m] = np.abs(sl).max(axis=0) * (MARGIN / 127.0)

    # --- effective expert weights: [2 strips, H, M*Adev] (no bias row) ---
    weff1 = np.empty((H, M * Adev), np.float32)
    for m in range(M):
        weff1[:, m * Adev : (m + 1) * Adev] = (Wdev[m] / scale[m][:, None]).T
    weff_bf = np.ascontiguousarray(
        np.broadcast_to(weff1.astype(BF16)[None], (2, H, M * Adev))
    )

    # --- pack per-core transposed activations (bf16, ramped groups) ---
    gbase = [sum(GROUPS[:g]) for g in range(len(GROUPS))]
    in_maps = []
    for i in range(NCORES):
        packed = np.zeros((R, H), BF16)
        for m in range(M):
            r0 = int(offs[m])
            packed[r0 : r0 + ncap[i][m]] = xb[core_idx[i][m][: caps[m]]]
        pv = packed.reshape(N_SUPER, 2, HALF, H)
        imap = {"weff": weff_bf}
        for g, gs in enumerate(GROUPS):
            imap[f"xat{g}"] = np.ascontiguousarray(
                pv[gbase[g] : gbase[g] + gs]
                .transpose(1, 3, 0, 2)
                .reshape(2, H, gs * HALF)
            )
        in_maps.append(imap)

    key = (R, caps, Adev)
    nc = _BUILD_CACHE.get(key)
    if nc is None:
        nc = _build(R, caps, Adev)
        _BUILD_CACHE[key] = nc

    # retry: rare transient NRT_EXEC_UNIT_UNRECOVERABLE on fresh NEFFs
    last_err = None
    for _attempt in range(3):
        try:
            res = run_bass_kernel_spmd(nc, in_maps, core_ids=list(range(NCORES)))
            break
        except Exception as e:  # noqa: BLE001
            last_err = e
    else:
        raise last_err
    LAST_RESULT = res

    # --- unpack: [n_pair, Adev, 2048] int8 -> rows, dequantize + bias ---
    out_kept = np.zeros((B, Adev), np.float32)
    bdev = bout[:, dev_cols]
    for i in range(NCORES):
        oc = np.asarray(res.results[i]["out"])
        rows = oc.transpose(0, 2, 1).reshape(-1, Adev)[:R]
        for m in range(M):
            r0 = int(offs[m])
            out_kept[core_idx[i][m][: caps[m]]] = (
                rows[r0 : r0 + ncap[i][m]].astype(np.float32) * scale[m][None, :]
                + bdev[m][None, :]
            )

    out_full = np.full((B, A), NEG, np.float32)
    out_full[:, dev_cols] = out_kept

    # --- host remainder: kept columns beyond the device's 128, plus the
    # few per-core cap-overflow rows (exact f32) ---
    for m in range(M):
        rows_m = np.nonzero(epoch_idx == m)[0]
        if len(rem_cols):
            out_full[rows_m[:, None], rem_cols[None, :]] = (
                x[rows_m] @ Wout[m][rem_cols].T + bout[m][rem_cols][None, :]
            )
        ov = np.concatenate(
            [core_idx[i][m][caps[m] :] for i in range(NCORES)]
        ).astype(np.int64)
        if len(ov):
            out_full[ov[:, None], dev_cols[None, :]] = (
                x[ov] @ Wout[m][dev_cols].T + bout[m][dev_cols][None, :]
            )

    return out_full.reshape(B, J, J)


# revision 35
# speedup vs baseline: 1.3175x; 1.1828x over previous
"""MoE-routing actor kernel for 8 Trainium2 NeuronCores.

Strategy (pure data parallel, expert-sorted, bf16 matmul, int8 output):
  - Host: fc1 trunk + relu on BLAS; rows dealt per-expert round-robin to the
    8 cores (shared SPMD graph); per-expert capacities trimmed so each core
    is exactly 32 supers of 1024 rows (overflow rows + mask columns beyond
    the 128 PSUM width are computed exactly on host).
  - Output is int8 with per-(expert, column) scales estimated from a 32k-row
    sample (margin 1.4); 1/scale is folded into the bf16 expert weights; the
    bias is applied on host during dequant (no ones-row -> less load
    traffic). err ~1.16e-2 unmasked (gate 2e-2).
  - Device (raw bacc): per 1024-row super, expert matmuls alternate PE
    partition strips 0/64 (concurrent sub-arrays). PSUM->int8 casts
    alternate DVE/ACT per super (~18us for 32 supers).
  - DMA reality (measured): the sync HWDGE queue sustains ~172 B/ns (~210
    solo), the gpsimd SWDGE queue ~124, the scalar queue ~50; they contend.
    The kernel is bound by the sync queue draining the int8 stores, so the
    schedule maximizes its productive window:
      * sync ring: weff as two 34-partition descs (no dead partitions),
        then supers 0-1 slot0, then all store pairs. The earlier the first
        cast, the earlier stores flow.
      * scalar ring: supers 0-1 slot1 + supers 2-3 (slow queue, early data).
      * gpsimd/SWDGE: released only after weff has landed (its traffic
        otherwise crushes the sync queue's small head transfers), then
        streams supers 4-31 in ramped groups; takes the s30 single store.
      * final pair stored as two singles, one per queue, to cut the tail.
  - No final completion wait: the walrus block-exit drain fences the rings
    during the semaphore-reset epilogue, hiding the last store's receipt.
"""

import os
import sys

sys.path.insert(0, "/opt/trn_rl_repo")

import numpy as np
import ml_dtypes

BF16 = ml_dtypes.bfloat16

B = 262144
NCORES = 8
J = 16
M = 12
H = 34
HP = H + 1  # transfers padded to 35 partitions: the HWDGE stripes a DMA's
            # per-partition descriptors over an engine count derived from
            # the partition span; 34-partition descs land on only 2 of 16
            # DMA engines (~50 B/ns), 35+ stripe wide. Row 34 is zeros.
S_DIM = 32
A = J * J
NEG = np.float32(-1.0e9)
SUPER = 1024
HALF = 512
NP = 4  # psum ring depth (supers)

GROUPS = (2, 2, 3, 4, 5, 6, 6, 4)  # supers per load group (g0 sync/scalar,
N_SUPER = sum(GROUPS)              # g1 scalar, g2+ gpsimd)

_BUILD_CACHE: dict = {}
LAST_RESULT = None


def _make_runs(caps, R):
    """Per 512-row half-chunk, the (expert, row0, row1) runs covering it."""
    offs = np.concatenate([[0], np.cumsum(caps)])
    assert offs[-1] == R
    runs = [[] for _ in range(R // HALF)]
    for m in range(len(caps)):
        lo, hi = int(offs[m]), int(offs[m + 1])
        if lo >= hi:
            continue
        for g in range(lo // HALF, (hi - 1) // HALF + 1):
            a = max(lo, g * HALF)
            b = min(hi, (g + 1) * HALF)
            if a < b:
                runs[g].append((m, a, b))
    return runs


def _build(R: int, caps: tuple, Adev: int):
    """Raw-bacc device graph: manual semaphores, static SBUF allocation."""
    from concourse import bacc, mybir

    n_half = R // HALF
    n_super = n_half // 2
    assert n_super == N_SUPER
    runs = _make_runs(list(caps), R)
    f32 = mybir.dt.float32
    bf16 = mybir.dt.bfloat16
    i8 = mybir.dt.int8
    nc = bacc.Bacc("TRN2", target_bir_lowering=False, debug=False)

    n_grp = len(GROUPS)
    gbase = [sum(GROUPS[:g]) for g in range(n_grp)]
    cbase = [b * HALF for b in gbase]
    sup2grp = {}
    for g in range(n_grp):
        for j in range(GROUPS[g]):
            sup2grp[gbase[g] + j] = (g, j)
    n_pair = n_super // 2

    xat_ds = [
        nc.declare_dram_parameter(f"xat{g}", [2, HP, GROUPS[g] * HALF], bf16,
                                  isOutput=False)
        for g in range(n_grp)
    ]
    # weff: [2 strips, H, M*Adev] -- two 34-partition descs, no dead rows
    weff_d = nc.declare_dram_parameter("weff", [2, HP, M * Adev], bf16,
                                       isOutput=False)
    out_d = nc.declare_dram_parameter(
        "out", [n_pair, Adev, 2 * SUPER], i8, isOutput=True
    )

    xa = nc.alloc_sbuf_tensor("xa_sb", [64 + HP, n_super * HALF], bf16)
    weff = nc.alloc_sbuf_tensor("weff_sb", [64 + HP, M * Adev], bf16)
    otb = nc.alloc_sbuf_tensor("ot_sb", [Adev, n_super * SUPER], i8)
    ots = [otb[:, s * SUPER : (s + 1) * SUPER] for s in range(n_super)]
    psos = [nc.alloc_psum_tensor(f"pso{k}", [Adev, SUPER], f32) for k in range(NP)]

    NSX = 4  # rotating input-load sems
    NSQ = 4  # rotating store sems
    sem_w = nc.alloc_semaphore("sem_w")
    sem_g0a = nc.alloc_semaphore("sem_g0a")
    sem_g0b = nc.alloc_semaphore("sem_g0b")
    sem_x = [nc.alloc_semaphore(f"sem_x{k}") for k in range(NSX)]
    sem_mm = nc.alloc_semaphore("sem_mm")
    sem_cv = nc.alloc_semaphore("sem_cv")
    sem_ca = nc.alloc_semaphore("sem_ca")
    sem_oe = [nc.alloc_semaphore(f"sem_oe{k}") for k in range(NSQ)]
    sem_og = nc.alloc_semaphore("sem_og")

    def gslice(g):
        return slice(cbase[g], cbase[g] + GROUPS[g] * HALF)

    # cast-engine assignment: DVE takes even supers, ACT takes odd supers
    dve_rank = {sc: sc // 2 + 1 for sc in range(0, n_super, 2)}
    act_rank = {sc: sc // 2 + 1 for sc in range(1, n_super, 2)}

    def wait_cast_done(eng, k):
        if k in dve_rank:
            eng.wait_ge(sem_cv, dve_rank[k])
        else:
            eng.wait_ge(sem_ca, act_rank[k])

    with nc.Block() as block:

        @block.gpsimd
        def _(g):
            for gi in range(2, n_grp):
                sx = sem_x[(gi - 1) % NSX]
                if gi - 1 >= NSX:
                    g.wait_ge(sx, 32 * ((gi - 1) // NSX))
                g.dma_start(xa[0:HP, gslice(gi)], xat_ds[gi][0]).then_inc(sx, 16)
                g.dma_start(xa[64 : 64 + HP, gslice(gi)], xat_ds[gi][1]).then_inc(
                    sx, 16
                )
            # single-super store for s30 on the swdge queue (parallel tail)
            wait_cast_done(g, n_super - 2)
            g.dma_start(
                out_d[n_pair - 1][:, 0:SUPER],
                otb[:, (n_super - 2) * SUPER : (n_super - 1) * SUPER],
            ).then_inc(sem_og, 16)

        @block.tensor
        def _(t):
            t.wait_ge(sem_w, 16)   # strip-0 weff; strip 1 gated below
            t.wait_ge(sem_g0a, 16)  # supers 0-1 slot0; slot1 gated below
            x0b_waited = False
            for sc in range(n_super):
                gi, j = sup2grp[sc]
                if j == 0 and gi > 0:
                    t.wait_ge(sem_x[(gi - 1) % NSX], 32 * ((gi - 1) // NSX + 1))
                if sc >= NP:
                    wait_cast_done(t, sc - NP)
                pso = psos[sc % NP]
                mms = []
                for h in range(2):
                    base = 0 if h == 0 else 64
                    for (m, a, b) in runs[sc * 2 + h]:
                        if h == 1 and not x0b_waited:
                            t.wait_ge(sem_g0b, 16)
                            t.wait_ge(sem_w, 32)  # strip-1 weff
                            x0b_waited = True
                        c0 = a - sc * SUPER
                        c1 = b - sc * SUPER
                        xcol = cbase[gi] + j * HALF
                        mms.append(
                            t.matmul(
                                pso[:, c0:c1],
                                weff[base : base + H, m * Adev : (m + 1) * Adev],
                                xa[
                                    base : base + H,
                                    xcol + c0 - h * HALF : xcol + c1 - h * HALF,
                                ],
                                start=True,
                                stop=True,
                            )
                        )
                mms[-1].then_inc(sem_mm, 1)

        @block.vector
        def _(v):
            for sc in range(0, n_super, 2):
                v.wait_ge(sem_mm, sc + 1)
                v.tensor_copy(ots[sc][:, :], psos[sc % NP][:, :]).then_inc(sem_cv, 1)

        @block.scalar
        def _(s):
            # group-0 high half + group 1 on the scalar HWDGE ring (slow
            # queue, but the data is early and small)
            s.dma_start(xa[64 : 64 + HP, gslice(0)], xat_ds[0][1]).then_inc(
                sem_g0b, 16
            )
            s.dma_start(xa[0:HP, gslice(1)], xat_ds[1][0]).then_inc(sem_x[0], 16)
            s.dma_start(xa[64 : 64 + HP, gslice(1)], xat_ds[1][1]).then_inc(
                sem_x[0], 16
            )
            for sc in range(1, n_super, 2):
                s.wait_ge(sem_mm, sc + 1)
                s.copy(ots[sc][:, :], psos[sc % NP][:, :]).then_inc(sem_ca, 1)

        @block.sync
        def _(sy):
            sy.dma_start(weff[0:HP, :], weff_d[0]).then_inc(sem_w, 16)
            sy.dma_start(weff[64 : 64 + HP, :], weff_d[1]).then_inc(sem_w, 16)
            sy.dma_start(xa[0:HP, gslice(0)], xat_ds[0][0]).then_inc(sem_g0a, 16)
            for p in range(n_pair - 1):
                wait_cast_done(sy, 2 * p)
                wait_cast_done(sy, 2 * p + 1)
                so = sem_oe[p % NSQ]
                if p >= NSQ:
                    sy.wait_ge(so, 16 * (p // NSQ))
                sy.dma_start(
                    out_d[p][:], otb[:, 2 * p * SUPER : (2 * p + 2) * SUPER]
                ).then_inc(so, 16)
            # final single-super store for s31 (s30 went out on swdge)
            wait_cast_done(sy, n_super - 1)
            sy.dma_start(
                out_d[n_pair - 1][:, SUPER : 2 * SUPER],
                otb[:, (n_super - 1) * SUPER : n_super * SUPER],
            ).then_inc(sem_oe[(n_pair - 1) % NSQ], 16)
            # no final completion wait: the block-exit drain fences the rings

    nc.compile()
    return nc


def kernel(states, epoch_idx, W1, b1, Wout, bout, mask):
    global LAST_RESULT
    from concourse.bass_utils import run_bass_kernel_spmd

    states = np.asarray(states, dtype=np.float32)
    epoch_idx = np.asarray(epoch_idx, dtype=np.int32)
    W1 = np.asarray(W1, dtype=np.float32)
    b1 = np.asarray(b1, dtype=np.float32)
    Wout = np.asarray(Wout, dtype=np.float32)
    bout = np.asarray(bout, dtype=np.float32)
    mask = np.asarray(mask, dtype=np.int32)

    keep = mask.reshape(A) != 0
    kept_cols = np.nonzero(keep)[0]
    Ak = int(len(kept_cols))
    if Ak == 0:
        return np.full((B, J, J), NEG, np.float32)
    Adev = min(Ak, 128)
    dev_cols = kept_cols[:Adev]
    rem_cols = kept_cols[Adev:]

    # --- shared trunk on host (tiny: ~0.6 GFLOP BLAS) ---
    x = np.maximum(states @ W1.T + b1[None, :], 0.0)  # [B, H] f32
    xb = x.astype(BF16)

    # --- route rows: per expert, deal round-robin across cores ---
    core_idx = [[None] * M for _ in range(NCORES)]
    for m in range(M):
        idx_m = np.nonzero(epoch_idx == m)[0]
        for i in range(NCORES):
            core_idx[i][m] = idx_m[i::NCORES]
    cnt = [[len(core_idx[i][m]) for m in range(M)] for i in range(NCORES)]
    caps = [max(cnt[i][m] for i in range(NCORES)) for m in range(M)]
    R = N_SUPER * SUPER
    excess = sum(caps) - R
    while excess > 0:
        m_big = max(range(M), key=lambda m: caps[m])
        d = min(excess, max(1, excess // M))
        caps[m_big] -= d
        excess -= d
    if excess < 0:
        caps[-1] += -excess
    caps = tuple(caps)
    offs = np.concatenate([[0], np.cumsum(caps)])
    ncap = [[min(cnt[i][m], caps[m]) for m in range(M)] for i in range(NCORES)]

    # --- int8 output scales from sampled bias-free logits ---
    SAMP = 32768
    MARGIN = 1.4
    rng = np.random.default_rng(12345)
    samp = rng.choice(B, SAMP, replace=False)
    Wdev = Wout[:, dev_cols, :]  # [M, Adev, H]
    scale = np.empty((M, Adev), np.float32)
    for m in range(M):
        rows_s = samp[epoch_idx[samp] == m]
        sl = x[rows_s] @ Wdev[m].T
        scale[m] = np.abs(sl).max(axis=0) * (MARGIN / 127.0)

    # --- effective expert weights: [2 strips, H, M*Adev] (no bias row) ---
    weff1 = np.zeros((HP, M * Adev), np.float32)
    for m in range(M):
        weff1[:H, m * Adev : (m + 1) * Adev] = (Wdev[m] / scale[m][:, None]).T
    weff_bf = np.ascontiguousarray(
        np.broadcast_to(weff1.astype(BF16)[None], (2, HP, M * Adev))
    )

    # --- pack per-core transposed activations (bf16, ramped groups) ---
    gbase = [sum(GROUPS[:g]) for g in range(len(GROUPS))]
    in_maps = []
    for i in range(NCORES):
        packed = np.zeros((R, HP), BF16)
        for m in range(M):
            r0 = int(offs[m])
            packed[r0 : r0 + ncap[i][m], :H] = xb[core_idx[i][m][: caps[m]]]
        pv = packed.reshape(N_SUPER, 2, HALF, HP)
        imap = {"weff": weff_bf}
        for g, gs in enumerate(GROUPS):
            imap[f"xat{g}"] = np.ascontiguousarray(
                pv[gbase[g] : gbase[g] + gs]
                .transpose(1, 3, 0, 2)
                .reshape(2, HP, gs * HALF)
            )
        in_maps.append(imap)

    key = (R, caps, Adev)
    nc = _BUILD_CACHE.get(key)
    if nc is None:
        nc = _build(R, caps, Adev)
        _BUILD_CACHE[key] = nc

    # retry: rare transient NRT_EXEC_UNIT_UNRECOVERABLE on fresh NEFFs
    last_err = None
    for _attempt in range(3):
        try:
            res = run_bass_kernel_spmd(nc, in_maps, core_ids=list(range(NCORES)))
            break
        except Exception as e:  # noqa: BLE001
            last_err = e
    else:
        raise last_err
    LAST_RESULT = res

    # --- unpack: [n_pair, Adev, 2048] int8 -> rows, dequantize + bias ---
    out_kept = np.zeros((B, Adev), np.float32)
    bdev = bout[:, dev_cols]
    for i in range(NCORES):
        oc = np.asarray(res.results[i]["out"])
        rows = oc.transpose(0, 2, 1).reshape(-1, Adev)[:R]
        for m in range(M):
            r0 = int(offs[m])
            out_kept[core_idx[i][m][: caps[m]]] = (
                rows[r0 : r0 + ncap[i][m]].astype(np.float32) * scale[m][None, :]
                + bdev[m][None, :]
            )

    out_full = np.full((B, A), NEG, np.float32)
    out_full[:, dev_cols] = out_kept

    # --- host remainder: kept columns beyond the device's 128, plus the
    # few per-core cap-overflow rows (exact f32) ---
    for m in range(M):
        rows_m = np.nonzero(epoch_idx == m)[0]
        if len(rem_cols):
            out_full[rows_m[:, None], rem_cols[None, :]] = (
                x[rows_m] @ Wout[m][rem_cols].T + bout[m][rem_cols][None, :]
            )
        ov = np.concatenate(
            [core_idx[i][m][caps[m] :] for i in range(NCORES)]
        ).astype(np.int64)
        if len(ov):
            out_full[ov[:, None], dev_cols[None, :]] = (
                x[ov] @ Wout[m][dev_cols].T + bout[m][dev_cols][None, :]
            )

    return out_full.reshape(B, J, J)


# revision 38
# speedup vs baseline: 1.3532x; 1.0271x over previous
"""MoE-routing actor kernel for 8 Trainium2 NeuronCores.

Strategy (pure data parallel, expert-sorted, bf16 matmul, int8 output):
  - Host: fc1 trunk + relu on BLAS; rows dealt per-expert round-robin to the
    8 cores (shared SPMD graph); per-expert capacities trimmed so each core
    is exactly 32 supers of 1024 rows (overflow rows + mask columns beyond
    the 128 PSUM width are computed exactly on host).
  - Output is int8 with per-(expert, column) scales estimated from a 32k-row
    sample (margin 1.4); 1/scale is folded into the bf16 expert weights; the
    bias is applied on host during dequant. err ~1.16e-2 unmasked (gate 2e-2).
  - Device (raw bacc): per 1024-row super, expert matmuls alternate PE
    partition strips 0/64 (concurrent sub-arrays). PSUM->int8 casts
    alternate DVE/ACT per super (~18us for 32 supers).
  - DMA reality (measured): queue throughput is set by how wide the DGE
    stripes a DMA's per-partition descriptors over the 16 DMA engines:
    HWDGE width grows with the desc's partition-line count (99 lines -> ~11
    engines, 35 -> 7, 34 -> 2!), SWDGE always stripes 16-wide. The sync
    HWDGE queue sustains ~170-210 B/ns, SWDGE ~124, scalar HWDGE ~40.
    The kernel is bound by draining the 4.2MB of int8 stores, so:
      * head: weff expert-0 piece (tiny, 99-line) then the rest of weff
        (99-line) on sync; supers 0-1 on the 16-wide SWDGE queue. First
        matmul ~11us, first cast ~11.8us -> stores start ~2.5us earlier
        than with the whole head on the sync ring.
      * loads: supers 2-3 ride the slow scalar ring (early, small); supers
        4-31 stream on SWDGE in ramped groups.
      * stores: sync takes 12 pairs + the s31 single; SWDGE takes pairs
        11/13 + the s30 single once its loads drain (parallel tail).
  - No final completion wait: the walrus block-exit drain fences the rings
    during the semaphore-reset epilogue, hiding the last store's receipt.
"""

import os
import sys

sys.path.insert(0, "/opt/trn_rl_repo")

import numpy as np
import ml_dtypes

BF16 = ml_dtypes.bfloat16

B = 262144
NCORES = 8
J = 16
M = 12
H = 34
HP = H + 1  # transfers padded to >=35 partitions (34-line descs stripe on
            # only 2 of 16 DMA engines); row 34 is zeros
S_DIM = 32
A = J * J
NEG = np.float32(-1.0e9)
SUPER = 1024
HALF = 512
NP = 4  # psum ring depth (supers)

GROUPS = (2, 2, 3, 4, 5, 6, 6, 4)  # supers per load group: g0 swdge-head,
N_SUPER = sum(GROUPS)              # g1 scalar, g2+ swdge stream

_BUILD_CACHE: dict = {}
LAST_RESULT = None


def _make_runs(caps, R):
    """Per 512-row half-chunk, the (expert, row0, row1) runs covering it."""
    offs = np.concatenate([[0], np.cumsum(caps)])
    assert offs[-1] == R
    runs = [[] for _ in range(R // HALF)]
    for m in range(len(caps)):
        lo, hi = int(offs[m]), int(offs[m + 1])
        if lo >= hi:
            continue
        for g in range(lo // HALF, (hi - 1) // HALF + 1):
            a = max(lo, g * HALF)
            b = min(hi, (g + 1) * HALF)
            if a < b:
                runs[g].append((m, a, b))
    return runs


def _build(R: int, caps: tuple, Adev: int):
    """Raw-bacc device graph: manual semaphores, static SBUF allocation."""
    from concourse import bacc, mybir

    n_half = R // HALF
    n_super = n_half // 2
    assert n_super == N_SUPER
    runs = _make_runs(list(caps), R)
    f32 = mybir.dt.float32
    bf16 = mybir.dt.bfloat16
    i8 = mybir.dt.int8
    nc = bacc.Bacc("TRN2", target_bir_lowering=False, debug=False)

    # experts needed by supers 0-1 -> tiny first weff piece
    eA = 1 + max(m for g in range(4) for (m, _, _) in runs[g])
    eA = min(eA, M)

    n_grp = len(GROUPS)
    gbase = [sum(GROUPS[:g]) for g in range(n_grp)]
    cbase = [b * HALF for b in gbase]
    sup2grp = {}
    for g in range(n_grp):
        for j in range(GROUPS[g]):
            sup2grp[gbase[g] + j] = (g, j)
    n_pair = n_super // 2

    xat_ds = [
        nc.declare_dram_parameter(f"xat{g}", [2, HP, GROUPS[g] * HALF], bf16,
                                  isOutput=False)
        for g in range(n_grp)
    ]
    # weff: [64+HP, M*Adev] with both strips (dead rows 35-63 zeroed) so the
    # descs are 99-line (wide engine striping)
    weff_d = nc.declare_dram_parameter("weff", [64 + HP, M * Adev], bf16,
                                       isOutput=False)
    out_d = nc.declare_dram_parameter(
        "out", [n_pair, Adev, 2 * SUPER], i8, isOutput=True
    )

    xa = nc.alloc_sbuf_tensor("xa_sb", [64 + HP, n_super * HALF], bf16)
    weff = nc.alloc_sbuf_tensor("weff_sb", [64 + HP, M * Adev], bf16)
    otb = nc.alloc_sbuf_tensor("ot_sb", [Adev, n_super * SUPER], i8)
    ots = [otb[:, s * SUPER : (s + 1) * SUPER] for s in range(n_super)]
    psos = [nc.alloc_psum_tensor(f"pso{k}", [Adev, SUPER], f32) for k in range(NP)]

    NSX = 4  # rotating input-load sems (groups 2..7)
    NSQ = 4  # rotating store sems
    sem_wa = nc.alloc_semaphore("sem_wa")  # weff experts [0,eA)
    sem_wb = nc.alloc_semaphore("sem_wb")  # weff experts [eA,M)
    sem_g0 = nc.alloc_semaphore("sem_g0")  # supers 0-1 (slot0 16, slot1 32)
    sem_g1 = nc.alloc_semaphore("sem_g1")  # supers 2-3 (scalar ring)
    sem_x = [nc.alloc_semaphore(f"sem_x{k}") for k in range(NSX)]
    sem_mm = nc.alloc_semaphore("sem_mm")
    sem_cv = nc.alloc_semaphore("sem_cv")
    sem_ca = nc.alloc_semaphore("sem_ca")
    sem_oe = [nc.alloc_semaphore(f"sem_oe{k}") for k in range(NSQ)]
    sem_og = nc.alloc_semaphore("sem_og")

    def gslice(g):
        return slice(cbase[g], cbase[g] + GROUPS[g] * HALF)

    # cast-engine assignment: DVE takes even supers, ACT takes odd supers
    dve_rank = {sc: sc // 2 + 1 for sc in range(0, n_super, 2)}
    act_rank = {sc: sc // 2 + 1 for sc in range(1, n_super, 2)}

    def wait_cast_done(eng, k):
        if k in dve_rank:
            eng.wait_ge(sem_cv, dve_rank[k])
        else:
            eng.wait_ge(sem_ca, act_rank[k])

    with nc.Block() as block:

        @block.gpsimd
        def _(g):
            # head supers 0-1 on the 16-wide SWDGE queue, slot0 then slot1
            g.dma_start(xa[0:HP, gslice(0)], xat_ds[0][0]).then_inc(sem_g0, 16)
            g.dma_start(xa[64 : 64 + HP, gslice(0)], xat_ds[0][1]).then_inc(
                sem_g0, 16
            )
            for gi in range(2, n_grp):
                i = gi - 2
                sx = sem_x[i % NSX]
                if i >= NSX:
                    g.wait_ge(sx, 32 * (i // NSX))
                g.dma_start(xa[0:HP, gslice(gi)], xat_ds[gi][0]).then_inc(sx, 16)
                g.dma_start(xa[64 : 64 + HP, gslice(gi)], xat_ds[gi][1]).then_inc(
                    sx, 16
                )
            # late stores once the load stream drains (parallel to sync)
            for s0, ns in ((18, 2), (22, 2), (26, 2), (n_super - 2, 1)):
                for s in range(s0, s0 + ns):
                    wait_cast_done(g, s)
                p = s0 // 2
                dst = out_d[p][:] if ns == 2 else out_d[p][:, 0:SUPER]
                g.dma_start(
                    dst, otb[:, s0 * SUPER : (s0 + ns) * SUPER]
                ).then_inc(sem_og, 16)

        @block.tensor
        def _(t):
            t.wait_ge(sem_wa, 16)
            t.wait_ge(sem_g0, 16)  # slot0; slot1 (h1 strip) gated below
            wb_waited = False
            x0b_waited = False
            g1_h1_waited = False
            for sc in range(n_super):
                gi, j = sup2grp[sc]
                if j == 0 and gi == 1:
                    t.wait_ge(sem_g1, 16)
                if j == 0 and gi >= 2:
                    i = gi - 2
                    t.wait_ge(sem_x[i % NSX], 32 * (i // NSX + 1))
                if sc >= NP:
                    wait_cast_done(t, sc - NP)
                pso = psos[sc % NP]
                mms = []
                for h in range(2):
                    base = 0 if h == 0 else 64
                    for (m, a, b) in runs[sc * 2 + h]:
                        if m >= eA and not wb_waited:
                            t.wait_ge(sem_wb, 16)
                            wb_waited = True
                        if h == 1 and not x0b_waited:
                            t.wait_ge(sem_g0, 32)
                            x0b_waited = True
                        if h == 1 and gi == 1 and not g1_h1_waited:
                            t.wait_ge(sem_g1, 32)
                            g1_h1_waited = True
                        c0 = a - sc * SUPER
                        c1 = b - sc * SUPER
                        xcol = cbase[gi] + j * HALF
                        mms.append(
                            t.matmul(
                                pso[:, c0:c1],
                                weff[base : base + H, m * Adev : (m + 1) * Adev],
                                xa[
                                    base : base + H,
                                    xcol + c0 - h * HALF : xcol + c1 - h * HALF,
                                ],
                                start=True,
                                stop=True,
                            )
                        )
                mms[-1].then_inc(sem_mm, 1)

        @block.vector
        def _(v):
            for sc in range(0, n_super, 2):
                v.wait_ge(sem_mm, sc + 1)
                v.tensor_copy(ots[sc][:, :], psos[sc % NP][:, :]).then_inc(sem_cv, 1)

        @block.scalar
        def _(s):
            # supers 2-3 on the (slow) scalar ring: small and early enough
            s.dma_start(xa[0:HP, gslice(1)], xat_ds[1][0]).then_inc(sem_g1, 16)
            s.dma_start(xa[64 : 64 + HP, gslice(1)], xat_ds[1][1]).then_inc(
                sem_g1, 16
            )
            for sc in range(1, n_super, 2):
                s.wait_ge(sem_mm, sc + 1)
                s.copy(ots[sc][:, :], psos[sc % NP][:, :]).then_inc(sem_ca, 1)

        @block.sync
        def _(sy):
            # weff: tiny expert-0..eA piece first (gates the first matmuls),
            # then the rest; both 99-line descs for wide striping
            sy.dma_start(weff[0 : 64 + HP, 0 : eA * Adev],
                         weff_d[:, 0 : eA * Adev]).then_inc(sem_wa, 16)
            if eA < M:
                sy.dma_start(weff[0 : 64 + HP, eA * Adev :],
                             weff_d[:, eA * Adev :]).then_inc(sem_wb, 16)
            pidx = [p for p in range(n_pair - 1) if p not in (9, 11, 13)]
            for i, p in enumerate(pidx):
                wait_cast_done(sy, 2 * p)
                wait_cast_done(sy, 2 * p + 1)
                so = sem_oe[i % NSQ]
                if i >= NSQ:
                    sy.wait_ge(so, 16 * (i // NSQ))
                sy.dma_start(
                    out_d[p][:], otb[:, 2 * p * SUPER : (2 * p + 2) * SUPER]
                ).then_inc(so, 16)
            # final single-super store for s31 (s30 went out on swdge)
            wait_cast_done(sy, n_super - 1)
            sy.dma_start(
                out_d[n_pair - 1][:, SUPER : 2 * SUPER],
                otb[:, (n_super - 1) * SUPER : n_super * SUPER],
            ).then_inc(sem_oe[len(pidx) % NSQ], 16)
            # no final completion wait: the block-exit drain fences the rings

    nc.compile()
    return nc


def kernel(states, epoch_idx, W1, b1, Wout, bout, mask):
    global LAST_RESULT
    from concourse.bass_utils import run_bass_kernel_spmd

    states = np.asarray(states, dtype=np.float32)
    epoch_idx = np.asarray(epoch_idx, dtype=np.int32)
    W1 = np.asarray(W1, dtype=np.float32)
    b1 = np.asarray(b1, dtype=np.float32)
    Wout = np.asarray(Wout, dtype=np.float32)
    bout = np.asarray(bout, dtype=np.float32)
    mask = np.asarray(mask, dtype=np.int32)

    keep = mask.reshape(A) != 0
    kept_cols = np.nonzero(keep)[0]
    Ak = int(len(kept_cols))
    if Ak == 0:
        return np.full((B, J, J), NEG, np.float32)
    Adev = min(Ak, 128)
    dev_cols = kept_cols[:Adev]
    rem_cols = kept_cols[Adev:]

    # --- shared trunk on host (tiny: ~0.6 GFLOP BLAS) ---
    x = np.maximum(states @ W1.T + b1[None, :], 0.0)  # [B, H] f32
    xb = x.astype(BF16)

    # --- route rows: per expert, deal round-robin across cores ---
    core_idx = [[None] * M for _ in range(NCORES)]
    for m in range(M):
        idx_m = np.nonzero(epoch_idx == m)[0]
        for i in range(NCORES):
            core_idx[i][m] = idx_m[i::NCORES]
    cnt = [[len(core_idx[i][m]) for m in range(M)] for i in range(NCORES)]
    caps = [max(cnt[i][m] for i in range(NCORES)) for m in range(M)]
    R = N_SUPER * SUPER
    excess = sum(caps) - R
    while excess > 0:
        m_big = max(range(M), key=lambda m: caps[m])
        d = min(excess, max(1, excess // M))
        caps[m_big] -= d
        excess -= d
    if excess < 0:
        caps[-1] += -excess
    caps = tuple(caps)
    offs = np.concatenate([[0], np.cumsum(caps)])
    ncap = [[min(cnt[i][m], caps[m]) for m in range(M)] for i in range(NCORES)]

    # --- int8 output scales from sampled bias-free logits ---
    SAMP = 32768
    MARGIN = 1.4
    rng = np.random.default_rng(12345)
    samp = rng.choice(B, SAMP, replace=False)
    Wdev = Wout[:, dev_cols, :]  # [M, Adev, H]
    scale = np.empty((M, Adev), np.float32)
    for m in range(M):
        rows_s = samp[epoch_idx[samp] == m]
        sl = x[rows_s] @ Wdev[m].T
        scale[m] = np.abs(sl).max(axis=0) * (MARGIN / 127.0)

    # --- effective expert weights: [64+HP, M*Adev], strips at 0 and 64 ---
    weff1 = np.zeros((64 + HP, M * Adev), np.float32)
    for m in range(M):
        weff1[:H, m * Adev : (m + 1) * Adev] = (Wdev[m] / scale[m][:, None]).T
    weff1[64 : 64 + H] = weff1[:H]
    weff_bf = weff1.astype(BF16)

    # --- pack per-core transposed activations (bf16, ramped groups) ---
    gbase = [sum(GROUPS[:g]) for g in range(len(GROUPS))]
    in_maps = []
    for i in range(NCORES):
        packed = np.zeros((R, HP), BF16)
        for m in range(M):
            r0 = int(offs[m])
            packed[r0 : r0 + ncap[i][m], :H] = xb[core_idx[i][m][: caps[m]]]
        pv = packed.reshape(N_SUPER, 2, HALF, HP)
        imap = {"weff": weff_bf}
        for g, gs in enumerate(GROUPS):
            imap[f"xat{g}"] = np.ascontiguousarray(
                pv[gbase[g] : gbase[g] + gs]
                .transpose(1, 3, 0, 2)
                .reshape(2, HP, gs * HALF)
            )
        in_maps.append(imap)

    key = (R, caps, Adev)
    nc = _BUILD_CACHE.get(key)
    if nc is None:
        nc = _build(R, caps, Adev)
        _BUILD_CACHE[key] = nc

    # retry: rare transient NRT_EXEC_UNIT_UNRECOVERABLE on fresh NEFFs
    last_err = None
    for _attempt in range(3):
        try:
            res = run_bass_kernel_spmd(nc, in_maps, core_ids=list(range(NCORES)))
            break
        except Exception as e:  # noqa: BLE001
            last_err = e
    else:
        raise last_err
    LAST_RESULT = res

    # --- unpack: [n_pair, Adev, 2048] int8 -> rows, dequantize + bias ---
    out_kept = np.zeros((B, Adev), np.float32)
    bdev = bout[:, dev_cols]
    for i in range(NCORES):
        oc = np.asarray(res.results[i]["out"])
        rows = oc.transpose(0, 2, 1).reshape(-1, Adev)[:R]
        for m in range(M):
            r0 = int(offs[m])
            out_kept[core_idx[i][m][: caps[m]]] = (
                rows[r0 : r0 + ncap[i][m]].astype(np.float32) * scale[m][None, :]
                + bdev[m][None, :]
            )

    out_full = np.full((B, A), NEG, np.float32)
    out_full[:, dev_cols] = out_kept

    # --- host remainder: kept columns beyond the device's 128, plus the
    # few per-core cap-overflow rows (exact f32) ---
    for m in range(M):
        rows_m = np.nonzero(epoch_idx == m)[0]
        if len(rem_cols):
            out_full[rows_m[:, None], rem_cols[None, :]] = (
                x[rows_m] @ Wout[m][rem_cols].T + bout[m][rem_cols][None, :]
            )
        ov = np.concatenate(
            [core_idx[i][m][caps[m] :] for i in range(NCORES)]
        ).astype(np.int64)
        if len(ov):
            out_full[ov[:, None], dev_cols[None, :]] = (
                x[ov] @ Wout[m][dev_cols].T + bout[m][dev_cols][None, :]
            )

    return out_full.reshape(B, J, J)


# revision 40
# speedup vs baseline: 1.3764x; 1.0171x over previous
"""MoE-routing actor kernel for 8 Trainium2 NeuronCores.

Strategy (pure data parallel, expert-sorted, bf16 matmul, int8 output):
  - Host: fc1 trunk + relu on BLAS; rows dealt per-expert round-robin to the
    8 cores (shared SPMD graph); per-expert capacities trimmed so each core
    is exactly 32 supers of 1024 rows (overflow rows + mask columns beyond
    the 128 PSUM width are computed exactly on host).
  - Output is int8 with per-(expert, column) scales estimated from a 32k-row
    sample (margin 1.4); 1/scale is folded into the bf16 expert weights; the
    bias is applied on host during dequant. err ~1.16e-2 unmasked (gate 2e-2).
  - Device (raw bacc): per 1024-row super, expert matmuls alternate PE
    partition strips 0/64 (concurrent sub-arrays). PSUM->int8 casts
    alternate DVE/ACT per super (~18us for 32 supers).
  - DMA reality (measured): queue throughput is set by how wide the DGE
    stripes a DMA's per-partition descriptors over the 16 DMA engines:
    HWDGE width grows with the desc's partition-line count (99 lines -> ~11
    engines, 35 -> 7, 34 -> 2!), SWDGE always stripes 16-wide. The sync
    HWDGE queue sustains ~170-210 B/ns, SWDGE ~124, scalar HWDGE ~40.
    The kernel is bound by draining the 4.2MB of int8 stores, so:
      * head: weff expert-0 piece (tiny, 99-line) then the rest of weff
        (99-line) on sync; supers 0-1 on the 16-wide SWDGE queue. First
        matmul ~11us, first cast ~11.8us -> stores start ~2.5us earlier
        than with the whole head on the sync ring.
      * loads: supers 2-3 ride the slow scalar ring (early, small); supers
        4-31 stream on SWDGE in ramped groups.
      * stores: sync takes 12 pairs + the s31 single; SWDGE takes pairs
        11/13 + the s30 single once its loads drain (parallel tail).
  - No final completion wait: the walrus block-exit drain fences the rings
    during the semaphore-reset epilogue, hiding the last store's receipt.
"""

import os
import sys

sys.path.insert(0, "/opt/trn_rl_repo")

import numpy as np
import ml_dtypes

BF16 = ml_dtypes.bfloat16

B = 262144
NCORES = 8
J = 16
M = 12
H = 34
HP = H + 1  # transfers padded to >=35 partitions (34-line descs stripe on
            # only 2 of 16 DMA engines); row 34 is zeros
S_DIM = 32
A = J * J
NEG = np.float32(-1.0e9)
SUPER = 1024
HALF = 512
NP = 4  # psum ring depth (supers)

GROUPS = (2, 2, 3, 4, 5, 6, 6, 4)  # supers per load group: g0 swdge-head,
N_SUPER = sum(GROUPS)              # g1 scalar, g2+ swdge stream

_BUILD_CACHE: dict = {}
LAST_RESULT = None


def _make_runs(caps, R):
    """Per 512-row half-chunk, the (expert, row0, row1) runs covering it."""
    offs = np.concatenate([[0], np.cumsum(caps)])
    assert offs[-1] == R
    runs = [[] for _ in range(R // HALF)]
    for m in range(len(caps)):
        lo, hi = int(offs[m]), int(offs[m + 1])
        if lo >= hi:
            continue
        for g in range(lo // HALF, (hi - 1) // HALF + 1):
            a = max(lo, g * HALF)
            b = min(hi, (g + 1) * HALF)
            if a < b:
                runs[g].append((m, a, b))
    return runs


def _build(R: int, caps: tuple, Adev: int):
    """Raw-bacc device graph: manual semaphores, static SBUF allocation."""
    from concourse import bacc, mybir

    n_half = R // HALF
    n_super = n_half // 2
    assert n_super == N_SUPER
    runs = _make_runs(list(caps), R)
    f32 = mybir.dt.float32
    bf16 = mybir.dt.bfloat16
    i8 = mybir.dt.int8
    nc = bacc.Bacc("TRN2", target_bir_lowering=False, debug=False)

    # experts needed by supers 0-1 -> tiny first weff piece
    eA = 1 + max(m for g in range(4) for (m, _, _) in runs[g])
    eA = min(eA, M)

    n_grp = len(GROUPS)
    gbase = [sum(GROUPS[:g]) for g in range(n_grp)]
    cbase = [b * HALF for b in gbase]
    sup2grp = {}
    for g in range(n_grp):
        for j in range(GROUPS[g]):
            sup2grp[gbase[g] + j] = (g, j)
    n_pair = n_super // 2

    xat_ds = [
        nc.declare_dram_parameter(f"xat{g}", [2, HP, GROUPS[g] * HALF], bf16,
                                  isOutput=False)
        for g in range(n_grp)
    ]
    # weff: [64+HP, M*Adev] with both strips (dead rows 35-63 zeroed) so the
    # descs are 99-line (wide engine striping)
    weff_d = nc.declare_dram_parameter("weff", [64 + HP, M * Adev], bf16,
                                       isOutput=False)
    out_d = nc.declare_dram_parameter(
        "out", [n_pair, Adev, 2 * SUPER], i8, isOutput=True
    )

    xa = nc.alloc_sbuf_tensor("xa_sb", [64 + HP, n_super * HALF], bf16)
    weff = nc.alloc_sbuf_tensor("weff_sb", [64 + HP, M * Adev], bf16)
    otb = nc.alloc_sbuf_tensor("ot_sb", [Adev, n_super * SUPER], i8)
    ots = [otb[:, s * SUPER : (s + 1) * SUPER] for s in range(n_super)]
    psos = [nc.alloc_psum_tensor(f"pso{k}", [Adev, SUPER], f32) for k in range(NP)]

    NSX = 4  # rotating input-load sems (groups 2..7)
    NSQ = 4  # rotating store sems
    sem_wa = nc.alloc_semaphore("sem_wa")  # weff experts [0,eA)
    sem_wb = nc.alloc_semaphore("sem_wb")  # weff experts [eA,M)
    sem_g0a = nc.alloc_semaphore("sem_g0a")  # supers 0-1 slot0 (sync)
    sem_g0b = nc.alloc_semaphore("sem_g0b")  # supers 0-1 slot1 (swdge)
    sem_g1 = nc.alloc_semaphore("sem_g1")  # supers 2-3 (scalar ring)
    sem_x = [nc.alloc_semaphore(f"sem_x{k}") for k in range(NSX)]
    sem_mm = nc.alloc_semaphore("sem_mm")
    sem_cv = nc.alloc_semaphore("sem_cv")
    sem_ca = nc.alloc_semaphore("sem_ca")
    sem_oe = [nc.alloc_semaphore(f"sem_oe{k}") for k in range(NSQ)]
    sem_og = nc.alloc_semaphore("sem_og")

    def gslice(g):
        return slice(cbase[g], cbase[g] + GROUPS[g] * HALF)

    # cast-engine assignment: DVE takes even supers, ACT takes odd supers
    dve_rank = {sc: sc // 2 + 1 for sc in range(0, n_super, 2)}
    act_rank = {sc: sc // 2 + 1 for sc in range(1, n_super, 2)}

    def wait_cast_done(eng, k):
        if k in dve_rank:
            eng.wait_ge(sem_cv, dve_rank[k])
        else:
            eng.wait_ge(sem_ca, act_rank[k])

    with nc.Block() as block:

        @block.gpsimd
        def _(g):
            # head slot1 (h1 strip) first: its ~2.4us SWDGE completion
            # receipt hides under the h0 matmuls (slot0 rides sync)
            g.dma_start(xa[64 : 64 + HP, gslice(0)], xat_ds[0][1]).then_inc(
                sem_g0b, 16
            )
            for gi in range(2, n_grp):
                i = gi - 2
                sx = sem_x[i % NSX]
                if i >= NSX:
                    g.wait_ge(sx, 32 * (i // NSX))
                g.dma_start(xa[0:HP, gslice(gi)], xat_ds[gi][0]).then_inc(sx, 16)
                g.dma_start(xa[64 : 64 + HP, gslice(gi)], xat_ds[gi][1]).then_inc(
                    sx, 16
                )
            # late stores once the load stream drains (parallel to sync)
            for s0, ns in ((22, 2), (26, 2), (n_super - 2, 1)):
                for s in range(s0, s0 + ns):
                    wait_cast_done(g, s)
                p = s0 // 2
                dst = out_d[p][:] if ns == 2 else out_d[p][:, 0:SUPER]
                g.dma_start(
                    dst, otb[:, s0 * SUPER : (s0 + ns) * SUPER]
                ).then_inc(sem_og, 16)

        @block.tensor
        def _(t):
            t.wait_ge(sem_wa, 16)
            t.wait_ge(sem_g0a, 16)  # slot0; slot1 (h1 strip) gated below
            wb_waited = False
            x0b_waited = False
            g1_h1_waited = False
            for sc in range(n_super):
                gi, j = sup2grp[sc]
                if j == 0 and gi == 1:
                    t.wait_ge(sem_g1, 16)
                if j == 0 and gi >= 2:
                    i = gi - 2
                    t.wait_ge(sem_x[i % NSX], 32 * (i // NSX + 1))
                if sc >= NP:
                    wait_cast_done(t, sc - NP)
                pso = psos[sc % NP]
                mms = []
                for h in range(2):
                    base = 0 if h == 0 else 64
                    for (m, a, b) in runs[sc * 2 + h]:
                        if m >= eA and not wb_waited:
                            t.wait_ge(sem_wb, 16)
                            wb_waited = True
                        if h == 1 and not x0b_waited:
                            t.wait_ge(sem_g0b, 16)
                            x0b_waited = True
                        if h == 1 and gi == 1 and not g1_h1_waited:
                            t.wait_ge(sem_g1, 32)
                            g1_h1_waited = True
                        c0 = a - sc * SUPER
                        c1 = b - sc * SUPER
                        xcol = cbase[gi] + j * HALF
                        mms.append(
                            t.matmul(
                                pso[:, c0:c1],
                                weff[base : base + H, m * Adev : (m + 1) * Adev],
                                xa[
                                    base : base + H,
                                    xcol + c0 - h * HALF : xcol + c1 - h * HALF,
                                ],
                                start=True,
                                stop=True,
                            )
                        )
                mms[-1].then_inc(sem_mm, 1)

        @block.vector
        def _(v):
            for sc in range(0, n_super, 2):
                v.wait_ge(sem_mm, sc + 1)
                v.tensor_copy(ots[sc][:, :], psos[sc % NP][:, :]).then_inc(sem_cv, 1)

        @block.scalar
        def _(s):
            # supers 2-3 on the (slow) scalar ring: small and early enough
            s.dma_start(xa[0:HP, gslice(1)], xat_ds[1][0]).then_inc(sem_g1, 16)
            s.dma_start(xa[64 : 64 + HP, gslice(1)], xat_ds[1][1]).then_inc(
                sem_g1, 16
            )
            for sc in range(1, n_super, 2):
                s.wait_ge(sem_mm, sc + 1)
                s.copy(ots[sc][:, :], psos[sc % NP][:, :]).then_inc(sem_ca, 1)

        @block.sync
        def _(sy):
            # weff: tiny expert-0..eA piece first (gates the first matmuls),
            # then the rest; both 99-line descs for wide striping
            sy.dma_start(weff[0 : 64 + HP, 0 : eA * Adev],
                         weff_d[:, 0 : eA * Adev]).then_inc(sem_wa, 16)
            sy.dma_start(xa[0:HP, gslice(0)], xat_ds[0][0]).then_inc(
                sem_g0a, 16
            )
            if eA < M:
                sy.dma_start(weff[0 : 64 + HP, eA * Adev :],
                             weff_d[:, eA * Adev :]).then_inc(sem_wb, 16)
            pidx = [p for p in range(n_pair - 1) if p not in (11, 13)]
            for i, p in enumerate(pidx):
                wait_cast_done(sy, 2 * p)
                wait_cast_done(sy, 2 * p + 1)
                so = sem_oe[i % NSQ]
                if i >= NSQ:
                    sy.wait_ge(so, 16 * (i // NSQ))
                sy.dma_start(
                    out_d[p][:], otb[:, 2 * p * SUPER : (2 * p + 2) * SUPER]
                ).then_inc(so, 16)
            # final single-super store for s31 (s30 went out on swdge)
            wait_cast_done(sy, n_super - 1)
            sy.dma_start(
                out_d[n_pair - 1][:, SUPER : 2 * SUPER],
                otb[:, (n_super - 1) * SUPER : n_super * SUPER],
            ).then_inc(sem_oe[len(pidx) % NSQ], 16)
            # no final completion wait: the block-exit drain fences the rings

    nc.compile()
    return nc


def kernel(states, epoch_idx, W1, b1, Wout, bout, mask):
    global LAST_RESULT
    from concourse.bass_utils import run_bass_kernel_spmd

    states = np.asarray(states, dtype=np.float32)
    epoch_idx = np.asarray(epoch_idx, dtype=np.int32)
    W1 = np.asarray(W1, dtype=np.float32)
    b1 = np.asarray(b1, dtype=np.float32)
    Wout = np.asarray(Wout, dtype=np.float32)
    bout = np.asarray(bout, dtype=np.float32)
    mask = np.asarray(mask, dtype=np.int32)

    keep = mask.reshape(A) != 0
    kept_cols = np.nonzero(keep)[0]
    Ak = int(len(kept_cols))
    if Ak == 0:
        return np.full((B, J, J), NEG, np.float32)
    Adev = min(Ak, 128)
    dev_cols = kept_cols[:Adev]
    rem_cols = kept_cols[Adev:]

    # --- shared trunk on host (tiny: ~0.6 GFLOP BLAS) ---
    x = np.maximum(states @ W1.T + b1[None, :], 0.0)  # [B, H] f32
    xb = x.astype(BF16)

    # --- route rows: per expert, deal round-robin across cores ---
    core_idx = [[None] * M for _ in range(NCORES)]
    for m in range(M):
        idx_m = np.nonzero(epoch_idx == m)[0]
        for i in range(NCORES):
            core_idx[i][m] = idx_m[i::NCORES]
    cnt = [[len(core_idx[i][m]) for m in range(M)] for i in range(NCORES)]
    caps = [max(cnt[i][m] for i in range(NCORES)) for m in range(M)]
    R = N_SUPER * SUPER
    excess = sum(caps) - R
    while excess > 0:
        m_big = max(range(M), key=lambda m: caps[m])
        d = min(excess, max(1, excess // M))
        caps[m_big] -= d
        excess -= d
    if excess < 0:
        caps[-1] += -excess
    caps = tuple(caps)
    offs = np.concatenate([[0], np.cumsum(caps)])
    ncap = [[min(cnt[i][m], caps[m]) for m in range(M)] for i in range(NCORES)]

    # --- int8 output scales from sampled bias-free logits ---
    SAMP = 32768
    MARGIN = 1.4
    rng = np.random.default_rng(12345)
    samp = rng.choice(B, SAMP, replace=False)
    Wdev = Wout[:, dev_cols, :]  # [M, Adev, H]
    scale = np.empty((M, Adev), np.float32)
    for m in range(M):
        rows_s = samp[epoch_idx[samp] == m]
        sl = x[rows_s] @ Wdev[m].T
        scale[m] = np.abs(sl).max(axis=0) * (MARGIN / 127.0)

    # --- effective expert weights: [64+HP, M*Adev], strips at 0 and 64 ---
    weff1 = np.zeros((64 + HP, M * Adev), np.float32)
    for m in range(M):
        weff1[:H, m * Adev : (m + 1) * Adev] = (Wdev[m] / scale[m][:, None]).T
    weff1[64 : 64 + H] = weff1[:H]
    weff_bf = weff1.astype(BF16)

    # --- pack per-core transposed activations (bf16, ramped groups) ---
    gbase = [sum(GROUPS[:g]) for g in range(len(GROUPS))]
    in_maps = []
    for i in range(NCORES):
        packed = np.zeros((R, HP), BF16)
        for m in range(M):
            r0 = int(offs[m])
            packed[r0 : r0 + ncap[i][m], :H] = xb[core_idx[i][m][: caps[m]]]
        pv = packed.reshape(N_SUPER, 2, HALF, HP)
        imap = {"weff": weff_bf}
        for g, gs in enumerate(GROUPS):
            imap[f"xat{g}"] = np.ascontiguousarray(
                pv[gbase[g] : gbase[g] + gs]
                .transpose(1, 3, 0, 2)
                .reshape(2, HP, gs * HALF)
            )
        in_maps.append(imap)

    key = (R, caps, Adev)
    nc = _BUILD_CACHE.get(key)
    if nc is None:
        nc = _build(R, caps, Adev)
        _BUILD_CACHE[key] = nc

    # retry: rare transient NRT_EXEC_UNIT_UNRECOVERABLE on fresh NEFFs
    last_err = None
    for _attempt in range(3):
        try:
            res = run_bass_kernel_spmd(nc, in_maps, core_ids=list(range(NCORES)))
            break
        except Exception as e:  # noqa: BLE001
            last_err = e
    else:
        raise last_err
    LAST_RESULT = res

    # --- unpack: [n_pair, Adev, 2048] int8 -> rows, dequantize + bias ---
    out_kept = np.zeros((B, Adev), np.float32)
    bdev = bout[:, dev_cols]
    for i in range(NCORES):
        oc = np.asarray(res.results[i]["out"])
        rows = oc.transpose(0, 2, 1).reshape(-1, Adev)[:R]
        for m in range(M):
            r0 = int(offs[m])
            out_kept[core_idx[i][m][: caps[m]]] = (
                rows[r0 : r0 + ncap[i][m]].astype(np.float32) * scale[m][None, :]
                + bdev[m][None, :]
            )

    out_full = np.full((B, A), NEG, np.float32)
    out_full[:, dev_cols] = out_kept

    # --- host remainder: kept columns beyond the device's 128, plus the
    # few per-core cap-overflow rows (exact f32) ---
    for m in range(M):
        rows_m = np.nonzero(epoch_idx == m)[0]
        if len(rem_cols):
            out_full[rows_m[:, None], rem_cols[None, :]] = (
                x[rows_m] @ Wout[m][rem_cols].T + bout[m][rem_cols][None, :]
            )
        ov = np.concatenate(
            [core_idx[i][m][caps[m] :] for i in range(NCORES)]
        ).astype(np.int64)
        if len(ov):
            out_full[ov[:, None], dev_cols[None, :]] = (
                x[ov] @ Wout[m][dev_cols].T + bout[m][dev_cols][None, :]
            )

    return out_full.reshape(B, J, J)
